# revision 80
# baseline (speedup 1.0000x reference)
"""Trainium2 Bass kernel for nn_AGTLayer (sparse/linear attention layer).

Reference per node b (N=32768 nodes, D=512, H=8 heads x HD=64):
    q = relu(h @ Wq), k = relu(h @ Wk), v = h @ Wv       (per-node [H, HD])
    S = q @ k^T  [H,H];  num = S @ v;  denom = rowsum(S)
    attn = num / denom[:, None]
    out = layernorm(h + attn.flat @ Wf) * gamma + beta
    returns (out, q, k, v)

Mapping (per core: 4096 nodes, 32 tiles of 128, IO batched 4 tiles/super):
  - bf16 device IO (casts on CPU); TensorEngine matmuls in bf16.
  - Strided node groups (g = b mod 8): a group's (head, node) column set is a
    single strided AP -> one 128x128 S matmul per 16 nodes; cross-node junk
    masked during the PSUM->SBUF copy.
  - num = S_masked^T @ v_group + ones-column denom; attn = num * recip(denom).
  - v group-gather and attn scatter via head-major DRAM bounce layouts
    (single-DMA each); attnT via one xbar transpose of a pair-row DRAM image.
  - 4-stage software pipeline over super-tiles (S0 load/transpose prefetch,
    S1 projections+transposes, S2 attention core, S3 Wf+LN+store) with a
    2-slot S1->S2 gap, so PE/DVE/ACT stay dense while DMA chains of younger
    stages are in flight.  Cost-model timeline (TimelineSim): ~359 us/core.
"""

import sys

if "/opt/trn_rl_repo" not in sys.path:
    sys.path.insert(0, "/opt/trn_rl_repo")

import numpy as np
import ml_dtypes

import concourse.bacc as bacc
import concourse.mybir as mybir
from concourse import tile
from concourse.bass_utils import run_bass_kernel_spmd
from concourse.alu_op_type import AluOpType

BF16 = mybir.dt.bfloat16
F32 = mybir.dt.float32
AF = mybir.ActivationFunctionType

N, D, H = 32768, 512, 8
HD = D // H  # 64
LN_EPS = 1e-5
N_CORES = 8
NS = N // N_CORES       # rows per core = 4096
P = 128
NT = NS // P            # 32 tiles per core
TB = 4                  # tiles per IO super-batch
NST = NT // TB
NGRP = P // 16          # 8 strided groups per tile
PI = (0, 2, 4, 6, 1, 3, 5, 7)   # head order in assembled transposed q/k


def build_kernel(n_iters: int = 1):
    nc = bacc.Bacc("TRN2", target_bir_lowering=False)

    h = nc.declare_dram_parameter("h", [NS, D], BF16, isOutput=False)
    wq = nc.declare_dram_parameter("wq", [D, D], BF16, isOutput=False)
    wk = nc.declare_dram_parameter("wk", [D, D], BF16, isOutput=False)
    wv = nc.declare_dram_parameter("wv", [D, D], BF16, isOutput=False)
    wf = nc.declare_dram_parameter("wf", [D, D], BF16, isOutput=False)
    gamma = nc.declare_dram_parameter("gamma", [1, D], BF16, isOutput=False)
    beta = nc.declare_dram_parameter("beta", [1, D], BF16, isOutput=False)
    mask = nc.declare_dram_parameter("mask", [P, P], BF16, isOutput=False)

    q_o = nc.declare_dram_parameter("q_o", [NS, D], BF16, isOutput=True)
    k_o = nc.declare_dram_parameter("k_o", [NS, D], BF16, isOutput=True)
    v_o = nc.declare_dram_parameter("v_o", [NS, D], BF16, isOutput=True)
    out_o = nc.declare_dram_parameter("out_o", [NS, D], BF16, isOutput=True)

    with tile.TileContext(nc) as tc:
        with (
            tc.tile_pool(name="const", bufs=1) as cpool,
            tc.tile_pool(name="io", bufs=2) as io,
            tc.tile_pool(name="ioh", bufs=6) as ioh,
            tc.tile_pool(name="work", bufs=2) as work,
            tc.tile_pool(name="small", bufs=10) as small,
            tc.tile_pool(name="ps", bufs=1, space="PSUM") as ps,
            tc.tile_pool(name="dram", bufs=2, space="DRAM") as dpool,
        ):
            # ---- static data ----
            w_sb = {}
            for name, src in (("wq", wq), ("wk", wk), ("wv", wv), ("wf", wf)):
                t = cpool.tile([P, 4 * D], BF16, name=f"{name}_sb")
                nc.sync.dma_start(
                    t[:].rearrange("p (c f) -> p c f", c=4),
                    src[:].rearrange("(c p) f -> p c f", c=4))
                w_sb[name] = t
            gamma_rep = cpool.tile([P, D], BF16, name="gamma_rep")
            nc.sync.dma_start(gamma_rep[:], gamma[:].partition_broadcast(P))
            beta_rep = cpool.tile([P, D], BF16, name="beta_rep")
            nc.sync.dma_start(beta_rep[:], beta[:].partition_broadcast(P))
            mask_sb = cpool.tile([P, 4 * P], BF16, name="mask_sb")
            nc.sync.dma_start(
                mask_sb[:].rearrange("p (r c) -> p r c", r=4),
                mask[:].unsqueeze(1).broadcast_to([P, 4, P]))
            ones_sb = cpool.tile([P, 1], BF16, name="ones_sb")
            nc.vector.memset(ones_sb[:], 1.0)
            eps_sb = cpool.tile([P, 1], F32, name="eps_sb")
            nc.vector.memset(eps_sb[:], float(LN_EPS))

            def stage0(st):
                """prefetch: load h super-tile and transpose it."""
                r0 = st * TB * P
                rows4 = slice(r0, r0 + TB * P)
                h4 = ioh.tile([P, TB * D], BF16, tag="h4", name="h4")
                nc.sync.dma_start(
                    h4[:].rearrange("p (tt f) -> p tt f", tt=TB),
                    h[rows4, :].rearrange("(tt p) f -> p tt f", tt=TB))
                hT4 = work.tile([P, TB * D], BF16, tag="hT4", bufs=3, name="hT4")
                nc.scalar.dma_start_transpose(
                    hT4[:].rearrange("p (cc b) -> p cc b", b=P), h4[:])
                return dict(h4=h4, hT4=hT4)

            def stage1(s0, st):
                """projections, q/k transposes, v bounce + gather."""
                h4, hT4 = s0["h4"], s0["hT4"]
                q4 = io.tile([P, TB * D], BF16, tag="q4", name="q4")
                k4 = io.tile([P, TB * D], BF16, tag="k4", name="k4")
                v4 = io.tile([P, TB * D], BF16, tag="v4", name="v4")
                vgs = []
                for tt in range(TB):
                    f0 = tt * D
                    q_ps = ps.tile([P, D], F32, tag="q_ps", name="q_ps")
                    k_ps = ps.tile([P, D], F32, tag="k_ps", name="k_ps")
                    v_ps = ps.tile([P, D], F32, tag="v_ps", name="v_ps")
                    for c in range(4):
                        lhsT = hT4[:, (tt * 4 + c) * P:(tt * 4 + c + 1) * P]
                        kw = dict(start=(c == 0), stop=(c == 3))
                        nc.tensor.matmul(q_ps[:], lhsT, w_sb["wq"][:, c * D:(c + 1) * D], **kw)
                        nc.tensor.matmul(k_ps[:], lhsT, w_sb["wk"][:, c * D:(c + 1) * D], **kw)
                        nc.tensor.matmul(v_ps[:], lhsT, w_sb["wv"][:, c * D:(c + 1) * D], **kw)
                    nc.scalar.activation(q4[:, f0:f0 + D], q_ps[:], AF.Relu)
                    nc.scalar.activation(k4[:, f0:f0 + D], k_ps[:], AF.Relu)
                    nc.scalar.activation(v4[:, f0:f0 + D], v_ps[:], AF.Copy)
                for dst, srct in ((q_o, q4), (k_o, k4), (v_o, v4)):
                    r4 = slice(st * TB * P, (st + 1) * TB * P)
                    nc.sync.dma_start(
                        dst[r4, :].rearrange("(tt p) f -> p tt f", tt=TB),
                        srct[:].rearrange("p (tt f) -> p tt f", tt=TB))
                # batched transposes: raw chunks (tt,c) at (tt*4+c)*128
                q_raw = work.tile([P, TB * D], BF16, tag="q_raw", bufs=3, name="q_raw")
                k_raw = work.tile([P, TB * D], BF16, tag="k_raw", bufs=3, name="k_raw")
                nc.scalar.dma_start_transpose(
                    q_raw[:].rearrange("p (cc b) -> p cc b", b=P), q4[:])
                nc.scalar.dma_start_transpose(
                    k_raw[:].rearrange("p (cc b) -> p cc b", b=P), k4[:])
                # assembled: [64, even-span (2048) | odd-span (2048)]
                qt = work.tile([64, 2 * TB * D], BF16, tag="qt", bufs=3, name="qt")
                kt = work.tile([64, 2 * TB * D], BF16, tag="kt", bufs=3, name="kt")
                for dst, raw in ((qt, q_raw), (kt, k_raw)):
                    nc.sync.dma_start(dst[:, 0:TB * D], raw[0:64, :])
                    nc.sync.dma_start(dst[:, TB * D:2 * TB * D], raw[64:128, :])
                for tt in range(TB):
                    f0 = tt * D
                    # v bounce: row (m'*128 + b) holds v[b, PI(m'), :]
                    v_dram2 = dpool.tile([8 * P, HD], BF16, tag="v_dram2",
                                         bufs=16, name="v_dram2")
                    vd_v = v_dram2[:].rearrange("(a c b) e -> a b c e", a=2, c=4)
                    v4_v = v4[:, f0:f0 + D].rearrange("b (c a e) -> a b c e", a=2, e=HD)
                    for a in range(2):
                        nc.sync.dma_start(vd_v[a], v4_v[a])
                    vg = work.tile([P, NGRP * HD], BF16, tag="vg", bufs=14, name="vg")
                    nc.sync.dma_start(
                        vg[:].rearrange("t (g e) -> t g e", g=8),
                        v_dram2[:].rearrange("(t g) e -> t g e", g=8))
                    vgs.append(vg)
                return dict(h4=h4, qt=qt, kt=kt, vgs=vgs)

            def stage2(s1):
                """S, mask, num/denom, scale, attn bounce, attnT."""
                sms, n_pss, d_pss = [], [], []
                # [64, (span, u=(tt,c,jj), b=g)] -> [64, b, span, u]
                kt_v = s1["kt"][:].rearrange("p (s u b) -> p b s u", s=2, b=8)
                qt_v = s1["qt"][:].rearrange("p (s u b) -> p b s u", s=2, b=8)
                for tt in range(TB):
                    u0 = tt * 64
                    s_ps = [ps.tile([P, 4 * P], F32, tag=f"s_ps{x}", name=f"s_ps{x}")
                            for x in (0, 1)]
                    for g in range(NGRP):
                        blk = s_ps[g // 4][:, (g % 4) * P:(g % 4 + 1) * P]
                        for sp in range(2):
                            nc.tensor.matmul(
                                blk[sp * 64:(sp + 1) * 64, :],
                                kt_v[:, g, sp, u0:u0 + 64],
                                qt_v[:, g, :, u0:u0 + 64],
                                start=True, stop=True)
                    sm = [work.tile([P, 4 * P], BF16, tag=f"sm{x}", bufs=4, name=f"sm{x}")
                          for x in (0, 1)]
                    for x in (0, 1):
                        nc.vector.tensor_tensor(sm[x][:], s_ps[x][:], mask_sb[:],
                                                AluOpType.mult)
                    n_ps = ps.tile([P, NGRP * HD], F32, tag="n_ps", name="n_ps")
                    d_ps = ps.tile([P, NGRP], F32, tag="d_ps", name="d_ps")
                    vg = s1["vgs"][tt]
                    for g in range(NGRP):
                        smg = sm[g // 4][:, (g % 4) * P:(g % 4 + 1) * P]
                        nc.tensor.matmul(n_ps[:, g * HD:(g + 1) * HD], smg,
                                         vg[:, g * HD:(g + 1) * HD],
                                         start=True, stop=True)
                        nc.tensor.matmul(d_ps[:, g:g + 1], smg, ones_sb[:],
                                         start=True, stop=True)
                    recip_t = small.tile([P, NGRP], F32, tag="recip", name="recip")
                    nc.vector.reciprocal(recip_t[:], d_ps[:])
                    attn_g = work.tile([P, D], BF16, tag="attn_g", bufs=4, name="attn_g")
                    nc.vector.tensor_tensor(
                        attn_g[:].rearrange("p (g e) -> p g e", g=NGRP),
                        n_ps[:].rearrange("p (g e) -> p g e", g=NGRP),
                        recip_t[:].unsqueeze(2).broadcast_to([P, NGRP, HD]),
                        AluOpType.mult)
                    # pair-row bounce: a_dram4 row (tt*512 + c*128 + jj'*8 + g)
                    # = [e(head 2c) | e(head 2c+1)] of node (jj'*8+g) in tile tt
                    if tt == 0:
                        a_dram = dpool.tile([TB * 4 * P, P], BF16, tag="a_dram",
                                            bufs=3, name="a_dram")
                    rr = slice(tt * 4 * P, (tt + 1) * 4 * P)
                    for a in range(2):
                        nc.gpsimd.dma_start(
                            a_dram[rr, a * HD:(a + 1) * HD].rearrange(
                                "(t g) e -> t g e", g=8),
                            attn_g[a * 64:(a + 1) * 64, :].rearrange(
                                "t (g e) -> t g e", g=8))
                attnT4 = work.tile([P, TB * D], BF16, tag="attnT4", bufs=3, name="attnT4")
                nc.scalar.dma_start_transpose(attnT4[:], a_dram[:])
                return dict(attnT4=attnT4, h4=s1["h4"])

            def stage3(s2, st):
                """Wf projection, layernorm, store."""
                h4 = s2["h4"]
                out4 = io.tile([P, TB * D], BF16, tag="out4", name="out4")
                attnT4 = s2["attnT4"]
                for tt in range(TB):
                    f0 = tt * D
                    fh_ps = ps.tile([P, D], F32, tag="fh_ps", name="fh_ps")
                    for c in range(4):
                        nc.tensor.matmul(
                            fh_ps[:], attnT4[:, (tt * 4 + c) * P:(tt * 4 + c + 1) * P],
                            w_sb["wf"][:, c * D:(c + 1) * D],
                            start=(c == 0), stop=(c == 3))
                    x_bf = work.tile([P, D], BF16, tag="x_bf", name="x_bf")
                    sum_x = small.tile([P, 1], F32, tag="sum_x", name="sum_x")
                    nc.vector.scalar_tensor_tensor(
                        x_bf[:], fh_ps[:], 1.0, h4[:, f0:f0 + D],
                        AluOpType.mult, AluOpType.add, accum_out=sum_x[:])
                    neg_mu = small.tile([P, 1], F32, tag="neg_mu", name="neg_mu")
                    nc.vector.tensor_scalar_mul(neg_mu[:], sum_x[:], -1.0 / D)
                    sq = work.tile([P, D], BF16, tag="sq", name="sq")
                    ssq = small.tile([P, 1], F32, tag="ssq", name="ssq")
                    nc.scalar.activation(sq[:], x_bf[:], AF.Square,
                                         bias=neg_mu[:], scale=1.0,
                                         accum_out=ssq[:])
                    stdv = small.tile([P, 1], F32, tag="stdv", name="stdv")
                    nc.scalar.activation(stdv[:], ssq[:], AF.Sqrt,
                                         bias=eps_sb[:], scale=1.0 / D)
                    rstd = small.tile([P, 1], F32, tag="rstd", name="rstd")
                    nc.vector.reciprocal(rstd[:], stdv[:])
                    tmp = work.tile([P, D], BF16, tag="tmp", name="tmp")
                    nc.vector.scalar_tensor_tensor(
                        tmp[:], x_bf[:], neg_mu[:], gamma_rep[:],
                        AluOpType.add, AluOpType.mult)
                    nc.vector.scalar_tensor_tensor(
                        out4[:, f0:f0 + D], tmp[:], rstd[:], beta_rep[:],
                        AluOpType.mult, AluOpType.add)
                r4 = slice(st * TB * P, (st + 1) * TB * P)
                nc.sync.dma_start(
                    out_o[r4, :].rearrange("(tt p) f -> p tt f", tt=TB),
                    out4[:].rearrange("p (tt f) -> p tt f", tt=TB))

            for _ in range(n_iters):
                s0_q, s1_q, s2_q = [], [], []
                for i in range(NST + 4):
                    if i < NST:
                        s0_q.append(stage0(i))
                    if i >= 4:
                        stage3(s2_q.pop(0), i - 4)
                    if 3 <= i < NST + 3:
                        s2_q.append(stage2(s1_q.pop(0)))
                    if 1 <= i < NST + 1:
                        s1_q.append(stage1(s0_q.pop(0), i - 1))

    nc.finalize()
    return nc


def _mask_np() -> np.ndarray:
    # mask[(m',jj), (n',jj')] = 1 iff jj == jj'
    m = np.kron(np.ones((8, 8), np.float32), np.eye(16, dtype=np.float32))
    return m.astype(ml_dtypes.bfloat16)


_NC_CACHE: dict[int, object] = {}


def _get_nc(n_iters: int = 1):
    if n_iters not in _NC_CACHE:
        _NC_CACHE[n_iters] = build_kernel(n_iters)
    return _NC_CACHE[n_iters]


def make_in_maps(h, Wq, Wk, Wv, Wf, ln_gamma, ln_beta):
    bf = ml_dtypes.bfloat16
    h_bf = np.ascontiguousarray(np.asarray(h, np.float32)).astype(bf)
    wq_bf = np.asarray(Wq, np.float32).astype(bf)
    wk_bf = np.asarray(Wk, np.float32).astype(bf)
    wv_bf = np.asarray(Wv, np.float32).astype(bf)
    wf_bf = np.asarray(Wf, np.float32).astype(bf)
    g_bf = np.asarray(ln_gamma, np.float32).reshape(1, D).astype(bf)
    b_bf = np.asarray(ln_beta, np.float32).reshape(1, D).astype(bf)
    mask = _mask_np()
    in_maps = []
    for i in range(N_CORES):
        in_maps.append({
            "h": h_bf[i * NS:(i + 1) * NS],
            "wq": wq_bf, "wk": wk_bf, "wv": wv_bf, "wf": wf_bf,
            "gamma": g_bf, "beta": b_bf, "mask": mask,
        })
    return in_maps


def kernel(h, Wq, Wk, Wv, Wf, ln_gamma, ln_beta):
    nc = _get_nc(1)
    in_maps = make_in_maps(h, Wq, Wk, Wv, Wf, ln_gamma, ln_beta)
    res = run_bass_kernel_spmd(nc, in_maps, core_ids=list(range(N_CORES)))
    outs = {}
    for name in ("out_o", "q_o", "k_o", "v_o"):
        outs[name] = np.concatenate(
            [np.asarray(res.results[i][name]).astype(np.float32)
             for i in range(N_CORES)], axis=0)
    out = outs["out_o"]
    q = outs["q_o"].reshape(N, H, HD)
    k = outs["k_o"].reshape(N, H, HD)
    v = outs["v_o"].reshape(N, H, HD)
    return (out, q, k, v)
